# revision 67
# baseline (speedup 1.0000x reference)
"""AttentionCritic Trainium2 kernel — 8-core SPMD, no collectives.

Fast path (zero q/k biases, which setup_inputs produces):
  S_h = q_h k_h^T / 12 = C Mh C^T / 12,  Mh = (Wq Wiq)_h (Wk Wik)_h^T  (host)
  C = [obs, action], obs folded via W_enc into the u-projection weights:
  uT = Mall^T C^T computed as  WuA^T aT + Wuh^T hT   (no k projection at all)
  E_h = exp(S_h/12);  D[i,h,j] = sum_k E_h[j,k] m[i,k];  R = m/max(D,1e-9)
  W[i,h,k] = m[i,k] * sum_j R[i,h,j] E_h[j,k];  ctx0[i,h] = sum_k W v0_h[k]
  VA = sum_h ctx0_h @ Wbig_h + n_i*nvec + bva   (rank-1 PE updates)
  Q = V + A - mean(A)

mask computed directly in transposed [j, i-local] form on DVE from per-core
broadcast rows (x, y, global index) and per-partition j coords; n_i via PE.
All matmul inputs bf16 (fp32 PSUM); DMAs split over 4 HWDGE queues.
General path (nonzero q/k biases): separate q/k projections, same phase B.
"""

import sys

for _p in ("/opt/trn_rl_repo",):
    if _p not in sys.path:
        sys.path.append(_p)

import contextlib

import numpy as np
import ml_dtypes

import concourse.bass as bass
import concourse.bacc as bacc
import concourse.mybir as mybir
from concourse.tile import TileContext
from concourse import bass_utils

N, HID, ACT, NH = 256, 128, 5, 4
D, E, HD = 144, 576, 144
NCORES = 8
SH = N // NCORES  # 32
F32 = mybir.dt.float32
BF16 = mybir.dt.bfloat16
SCALE = 1.0 / 12.0
QKM = [(0, 128), (128, 128), (256, 128), (384, 128), (512, 128)]

# blobW bf16 [128, 768]: WuA-pad (tails block | m0..m3) id128
W_COLS = 768
# blobA bf16 [128, 528]: hT(256) wenc(16) aT(256)
A_COLS = 528
# fside f32 [128, 6]: negx0 negy0 negx1 negy1 jidx0 jidx1
FS_COLS = 6
# blobS bf16 [16, 660]: MallO-pad(640) WvQO(20)
S_COLS = 660
# blobF bf16 [128, 126]: WvQA(20) nvecQ+bvaQ row0(10) crow-replicated(96)
F_NB = 20
F_COLS = 126
N_WARM = 24
# uT tile order: tails first, then head mains
QKM5 = [(0, 128), (128, 128), (256, 128), (384, 128), (512, 128)]


def _build_fast():
    nc = bacc.Bacc(target_bir_lowering=False)

    def dp(name, shape, dtype, isOutput=False):
        return nc.declare_dram_parameter(name, shape, dtype, isOutput)

    blobA_d = dp("blobA", [128, A_COLS], BF16)
    blobW_d = dp("blobW", [128, W_COLS], BF16)
    blobS_d = dp("blobS", [16, S_COLS], BF16)
    blobF_d = dp("blobF", [128, F_COLS], BF16)
    fside_d = dp("fside", [128, FS_COLS], F32)
    out_d = dp("out", [SH, ACT], F32, isOutput=True)

    with TileContext(nc) as tc:
        with contextlib.ExitStack() as ctx:
            wp = ctx.enter_context(tc.tile_pool(name="wp", bufs=1))
            pp = ctx.enter_context(tc.tile_pool(name="pp", bufs=6, space="PSUM"))
            ptt = ctx.enter_context(tc.tile_pool(name="ptt", bufs=2,
                                                 space="PSUM"))

            def wt(shape, tag, dtype=BF16):
                return wp.tile(shape, dtype, tag=tag, name=tag)

            def ps(shape):
                return pp.tile(shape, F32, tag="mm", name="mm")

            # ---------- DMAs: 2 HWDGE queues + SWDGE, critical-first -------
            blobA = wt([128, A_COLS], "blobA")
            blobW = wt([128, W_COLS], "blobW")
            blobS = wt([16, S_COLS], "blobS")
            blobF = wt([128, F_COLS], "blobF")
            fside = wt([128, FS_COLS], "fside", F32)
            # sync: aT, WuA tails+m0, WuA m1
            nc.sync.dma_start(out=blobA[:, 272:528], in_=blobA_d[:, 272:528])
            nc.sync.dma_start(out=blobW[:, 0:256], in_=blobW_d[:, 0:256])
            nc.sync.dma_start(out=blobW[:, 256:384], in_=blobW_d[:, 256:384])
            # scalar: fside, hT+wenc, blobS, WuA m2+m3+id128, blobF(+crow)
            nc.scalar.dma_start(out=fside, in_=fside_d[:, :])
            nc.scalar.dma_start(out=blobA[:, 0:272], in_=blobA_d[:, 0:272])
            nc.scalar.dma_start(out=blobS, in_=blobS_d[:, :])
            nc.scalar.dma_start(out=blobW[:, 384:768], in_=blobW_d[:, 384:768])
            nc.scalar.dma_start(out=blobF, in_=blobF_d[:, :])
            xibc = blobF[:, 30:30 + SH]
            yibc = blobF[:, 30 + SH:30 + 2 * SH]
            idbc = blobF[:, 30 + 2 * SH:30 + 3 * SH]

            # ---------- tile views ----------
            hT = blobA[:, 0:256]
            wenc = blobA[:, 256:272]
            aT = blobA[:, 272:528]
            id128 = blobW[:, 640:768]
            wuO = blobS[:, 0:640]               # MallO-pad [16, 640]
            wvqO = blobS[:, 640:660]            # WvQO [16, 20]
            wvqA = blobF[:, 0:20]               # WvQA [128, 20]
            nvec_r = blobF[0:1, F_NB:F_NB + 5]
            bva_r = blobF[0:1, F_NB + 5:F_NB + 10]
            negx = [fside[:, 0:1], fside[:, 2:3]]
            negy = [fside[:, 1:2], fside[:, 3:4]]
            jidx = [fside[:, 4:5], fside[:, 5:6]]

            # ---------- PE warmup during the DMA wait ----------
            wup = wt([128, 128], "wup")
            nc.vector.memset(wup, 0.0)
            for _ in range(N_WARM):
                pw = ps([128, 128])
                nc.tensor.matmul(pw, wup, wup, start=True, stop=True)

            # ---------- obsT = W_enc^T hT  [16, 256] ----------
            p0 = ps([16, N])
            nc.tensor.matmul(p0, wenc, hT, start=True, stop=True)
            obsT = wt([16, N], "obsT")
            nc.vector.tensor_copy(out=obsT, in_=p0)

            # ---------- uT = Mall^T C^T: action chunk + obs chunk ----------
            E_t = [[None, None] for _ in range(NH)]
            ET_t = [[None, None] for _ in range(NH)]
            uTm = [None] * 4
            uTt = [None] * 4
            vq_t = []

            def emit_vq():
                for nt in range(2):
                    p = ps([128, 20])
                    nc.tensor.matmul(p, aT[:, nt * 128:(nt + 1) * 128], wvqA,
                                     start=True, stop=False)
                    nc.tensor.matmul(p, obsT[:, nt * 128:(nt + 1) * 128],
                                     wvqO, start=False, stop=True)
                    t = wt([128, 20], f"vq{nt}")
                    nc.any.tensor_copy(out=t, in_=p)
                    vq_t.append(t)

            for mi, (ms, ml) in enumerate(QKM5):
                p = ps([ml, N])
                nc.tensor.matmul(p, blobW[:, ms:ms + ml], aT,
                                 start=True, stop=False)
                nc.tensor.matmul(p, wuO[:, ms:ms + ml], obsT,
                                 start=False, stop=True)
                if mi == 0:
                    for h in range(4):
                        t = wt([16, N], f"uTt{h}")
                        nc.vector.tensor_copy(out=t,
                                              in_=p[32 * h:32 * h + 16, :])
                        uTt[h] = t
                else:
                    t = wt([128, N], f"uTm{mi - 1}")
                    nc.any.tensor_copy(out=t, in_=p)
                    uTm[mi - 1] = t

            # ---------- mcT[km][j, i-local] directly on DVE ----------
            mcT = []
            for km in range(2):
                dx = wt([128, SH], f"dx{km}")
                dy = wt([128, SH], f"dy{km}")
                nc.vector.tensor_scalar(dx, xibc, negx[km], None,
                                        mybir.AluOpType.add)
                nc.vector.tensor_scalar(dy, yibc, negy[km], None,
                                        mybir.AluOpType.add)
                dx2 = wt([128, SH], f"dx2{km}")
                dy2 = wt([128, SH], f"dy2{km}")
                nc.vector.tensor_tensor(dx2, dx, dx, mybir.AluOpType.mult)
                nc.vector.tensor_tensor(dy2, dy, dy, mybir.AluOpType.mult)
                nc.vector.tensor_scalar(dx, dx2, 16.0, None,
                                        mybir.AluOpType.is_le)
                nc.vector.tensor_scalar(dy, dy2, 4.0, None,
                                        mybir.AluOpType.is_le)
                up = wt([128, SH], f"up{km}")
                nc.vector.tensor_scalar(up, idbc, jidx[km], None,
                                        mybir.AluOpType.is_lt)
                pm = wt([128, SH], f"pm{km}")
                nc.vector.tensor_tensor(pm, dx, dy, mybir.AluOpType.mult)
                mk = wt([128, SH], f"mcT{km}")
                nc.vector.tensor_tensor(mk, pm, up, mybir.AluOpType.mult)
                mcT.append(mk)


            # ---------- S_h -> E_h (bf16); E_h^T via PE transpose ----------
            def emit_S(h):
                for mj in range(2):
                    sl = slice(mj * 128, (mj + 1) * 128)
                    pS = ps([128, N])
                    nc.tensor.matmul(pS, uTm[h][:, sl], aT,
                                     start=True, stop=False)
                    nc.tensor.matmul(pS, uTt[h][:, sl], obsT,
                                     start=False, stop=True)
                    Eh = wt([128, N], f"E{h}_{mj}")
                    nc.scalar.activation(Eh, pS,
                                         mybir.ActivationFunctionType.Exp,
                                         scale=SCALE)
                    E_t[h][mj] = Eh

            ncopy = [0]

            def emit_tr(h):
                for kb in range(2):
                    ETh = wt([128, N], f"ET{h}_{kb}")
                    for mj in range(2):
                        pt = ptt.tile([128, 128], BF16, tag="tt", name="tt")
                        nc.tensor.transpose(
                            pt, E_t[h][mj][:, kb * 128:(kb + 1) * 128], id128)
                        dst = ETh[:, mj * 128:(mj + 1) * 128]
                        if ncopy[0] % 2 == 0:
                            nc.vector.tensor_copy(out=dst, in_=pt)
                        else:
                            nc.scalar.activation(
                                dst, pt, mybir.ActivationFunctionType.Copy)
                        ncopy[0] += 1
                    ET_t[h][kb] = ETh

            emit_S(0)
            emit_S(1)
            emit_tr(0)
            emit_S(2)
            emit_tr(1)
            emit_S(3)
            emit_tr(2)
            emit_tr(3)
            emit_vq()


            # ---------- n_i^T [1, 32] ----------
            ones_t = wt([128, 1], "ones_t")
            nc.vector.memset(ones_t, 1.0)
            pn = ps([1, SH])
            for c in range(2):
                nc.tensor.matmul(pn, ones_t, mcT[c],
                                 start=(c == 0), stop=(c == 1))
            n_bf = wt([1, SH], "n_bf")
            nc.any.tensor_copy(out=n_bf, in_=pn)

            # ---------- R^T (bf16 chain), W^T, split-Q ----------
            ones_r = wt([1, SH], "ones_r")
            nc.vector.memset(ones_r, 1.0)
            RT = {}
            for h in range(NH):
                for jm in range(2):
                    p = ps([128, SH])
                    for kc in range(2):
                        nc.tensor.matmul(
                            p, ET_t[h][kc][:, jm * 128:(jm + 1) * 128],
                            mcT[kc], start=(kc == 0), stop=(kc == 1))
                    rtf = wt([128, SH], f"RTf{h}_{jm}", F32)
                    nc.vector.tensor_scalar(rtf, p, 1e-9, None,
                                            mybir.AluOpType.max)
                    with nc.allow_low_precision(reason="attn renorm"):
                        nc.vector.reciprocal(rtf, rtf)
                    rt = wt([128, SH], f"RT{h}_{jm}")
                    nc.vector.tensor_tensor(rt, rtf, mcT[jm],
                                            mybir.AluOpType.mult)
                    RT[(h, jm)] = rt
            WT = {}
            for h in range(NH):
                for km in range(2):
                    p = ps([128, SH])
                    for jc in range(2):
                        nc.tensor.matmul(
                            p, E_t[h][jc][:, km * 128:(km + 1) * 128],
                            RT[(h, jc)], start=(jc == 0), stop=(jc == 1))
                    wtl = wt([128, SH], f"WT{h}_{km}")
                    nc.vector.tensor_tensor(wtl, p, mcT[km],
                                            mybir.AluOpType.mult)
                    WT[(h, km)] = wtl
            pQa = ps([SH, ACT])
            for g, (h, km) in enumerate([(0, 0), (0, 1), (1, 0), (1, 1)]):
                nc.tensor.matmul(pQa, WT[(h, km)],
                                 vq_t[km][:, 5 * h:5 * h + 5],
                                 start=(g == 0), stop=False)
            nc.tensor.matmul(pQa, n_bf, nvec_r, start=False, stop=False)
            nc.tensor.matmul(pQa, ones_r, bva_r, start=False, stop=True)
            Qa_sb = wt([SH, ACT], "Qasb", F32)
            nc.scalar.activation(Qa_sb, pQa, mybir.ActivationFunctionType.Copy)
            pQb = ps([SH, ACT])
            for g, (h, km) in enumerate([(2, 0), (2, 1), (3, 0), (3, 1)]):
                nc.tensor.matmul(pQb, WT[(h, km)],
                                 vq_t[km][:, 5 * h:5 * h + 5],
                                 start=(g == 0), stop=(g == 3))
            Q_sb = wt([SH, ACT], "Qsb", F32)
            nc.vector.tensor_tensor(Q_sb, pQb, Qa_sb, mybir.AluOpType.add)
            nc.sync.dma_start(out=out_d[:, :], in_=Q_sb, single_packet=True)

    nc.compile()
    return nc


_NC_CACHE = {}
BF = ml_dtypes.bfloat16


def _make_in_maps_fast(inputs):
    f32 = np.float32
    g = lambda k: np.asarray(inputs[k], dtype=f32)

    hidden, action = g("hidden_state_n"), g("action_n")
    state = np.asarray(inputs["state_n"]).astype(np.int32)
    W_enc = g("W_enc")

    Wqf = g("Wq") @ g("Wiq")                    # [144, 576]
    Wkf = g("Wk") @ g("Wik")
    # Mall[:, 144h:144h+144] = Qh @ Kh^T  over C-features
    Mall = np.concatenate(
        [Wqf[:, 144 * h:144 * h + 144] @ Wkf[:, 144 * h:144 * h + 144].T
         for h in range(4)], axis=1)            # [144, 576]

    Wvf = g("Wv") @ g("Wiv")
    bvf = g("b_enc") @ Wvf[0:16] + g("bv") @ g("Wiv") + g("biv")   # [576]

    Wva6 = np.concatenate([g("W_val").reshape(D, 1),
                           g("W_adv").reshape(D, ACT)], axis=1)    # [144,6]
    WoWO = g("Wo_proj") @ g("W_O")                                 # [576,144]
    Wbig = WoWO @ Wva6                                             # [576,6]
    nvec = bvf @ Wbig + (g("bo_proj") @ g("W_O")) @ Wva6           # [6]
    bva6 = np.concatenate([g("b_val").reshape(1), g("b_adv")])     # [6]
    # dueling head folded in: Q = V + A - mean(A)  ==  VA6 @ T6
    T6 = np.zeros((6, ACT), f32)
    T6[0, :] = 1.0
    T6[1:6, :] = np.eye(ACT, dtype=f32) - 1.0 / ACT
    Wbig = Wbig @ T6                                               # [576,5]
    nvec = nvec @ T6
    bva6 = bva6 @ T6
    # v fused all the way into Q-space: WvQ [144, 20]
    WvQ = np.concatenate(
        [Wvf[:, 144 * h:144 * h + 144] @ Wbig[144 * h:144 * (h + 1)]
         for h in range(4)], axis=1)                               # [144, 20]

    def padu(w):  # [*, 576] head-blocks [obs16|act128] -> [*, 640]
        mains = [w[:, 144 * h + 16:144 * h + 144] for h in range(4)]
        z = np.zeros((w.shape[0], 16), f32)
        tails = []
        for h in range(4):
            tails += [w[:, 144 * h:144 * h + 16], z]
        return np.concatenate(tails + mains, axis=1)   # tails block first

    blobA = None  # assembled below with fside columns
    blobW = np.concatenate([padu(Mall[16:144]),
                            np.eye(128, dtype=f32)], axis=1)       # [128, 768]
    blobS = np.concatenate([padu(Mall[0:16]), WvQ[0:16]], axis=1)  # [16, 660]
    state_f = state.astype(f32)
    fside = np.zeros((128, 6), f32)
    fside[:, 0] = -state_f[0:128, 0]
    fside[:, 1] = -state_f[0:128, 1]
    fside[:, 2] = -state_f[128:256, 0]
    fside[:, 3] = -state_f[128:256, 1]
    fside[:, 4] = np.arange(128, dtype=f32)
    fside[:, 5] = np.arange(128, 256, dtype=f32)
    blobA = np.concatenate([np.ascontiguousarray(hidden.T), W_enc,
                            np.ascontiguousarray(action.T)], axis=1)

    nbrow = np.zeros((128, 10), f32)
    nbrow[0, 0:5] = nvec
    nbrow[0, 5:10] = bva6

    shared = {
        "blobA": blobA.astype(BF),
        "blobW": np.ascontiguousarray(blobW).astype(BF),
        "blobS": np.ascontiguousarray(blobS).astype(BF),
        "fside": fside,
    }
    in_maps = []
    for c in range(NCORES):
        crow = np.concatenate([state_f[c * SH:(c + 1) * SH, 0],
                               state_f[c * SH:(c + 1) * SH, 1],
                               np.arange(c * SH, (c + 1) * SH, dtype=f32)])
        bF = np.concatenate([WvQ[16:144], nbrow,
                             np.tile(crow, (128, 1))], axis=1)     # [128,126]
        m = dict(shared)
        m["blobF"] = np.ascontiguousarray(bF).astype(BF)
        in_maps.append(m)
    return in_maps


def _zero_qk_bias(inputs):
    return all(not np.any(np.asarray(inputs[k]))
               for k in ("bq", "bk", "biq", "bik"))


def kernel(**inputs):
    if not _zero_qk_bias(inputs):
        return _kernel_general(inputs)
    if "fast" not in _NC_CACHE:
        _NC_CACHE["fast"] = _build_fast()
    nc = _NC_CACHE["fast"]
    in_maps = _make_in_maps_fast(inputs)
    res = bass_utils.run_bass_kernel_spmd(nc, in_maps,
                                          core_ids=list(range(NCORES)))
    return np.concatenate([res.results[c]["out"] for c in range(NCORES)],
                          axis=0)


# ======================= general path (nonzero q/k biases) ==================
EPg = 640
QKMg = [(0, 128), (128, 128), (256, 128), (384, 128), (512, 64), (576, 64)]
GW_QA, GW_QH, GW_KA, GW_KH, GW_VA, GW_VH = (0, EPg, 2 * EPg, 3 * EPg,
                                            4 * EPg, 4 * EPg + E)
GW_COLS = 4 * EPg + 2 * E
GF_SEL, GF_WB = 0, 64
GF_COLS = GF_WB + 48
GFS_NEG, GFS_BQ, GFS_BK = 0, 4, 10
GFS_COLS = 16


def _build_general():
    nc = bacc.Bacc(target_bir_lowering=False)

    def dp(name, shape, dtype, isOutput=False):
        return nc.declare_dram_parameter(name, shape, dtype, isOutput)

    blobA_d = dp("blobA", [128, 512], BF16)
    blobW_d = dp("blobW", [128, GW_COLS], BF16)
    blobF_d = dp("blobF", [128, GF_COLS], BF16)
    fside_d = dp("fside", [128, GFS_COLS], F32)
    srows_d = dp("srows", [2, N], BF16)
    nb2_d = dp("nb2", [1, 12], F32)
    out_d = dp("out", [SH, ACT], F32, isOutput=True)

    with TileContext(nc) as tc:
        with contextlib.ExitStack() as ctx:
            wp = ctx.enter_context(tc.tile_pool(name="wp", bufs=1))
            pp = ctx.enter_context(tc.tile_pool(name="pp", bufs=7, space="PSUM"))

            def wt(shape, tag, dtype=BF16):
                return wp.tile(shape, dtype, tag=tag, name=tag)

            def ps(shape):
                return pp.tile(shape, F32, tag="mm", name="mm")

            dma = nc.sync.dma_start
            sdma = nc.scalar.dma_start
            gdma = nc.gpsimd.dma_start

            blobA = wt([128, 512], "blobA")
            dma(out=blobA, in_=blobA_d[:, :])
            blobW = wt([128, GW_COLS], "blobW")
            dma(out=blobW[:, 0:2 * EPg], in_=blobW_d[:, 0:2 * EPg])
            fside = wt([128, GFS_COLS], "fside", F32)
            sdma(out=fside, in_=fside_d[:, :])
            sdma(out=blobW[:, 2 * EPg:4 * EPg], in_=blobW_d[:, 2 * EPg:4 * EPg])
            dma(out=blobW[:, 4 * EPg:GW_COLS], in_=blobW_d[:, 4 * EPg:GW_COLS])
            blobF = wt([128, GF_COLS], "blobF")
            sdma(out=blobF, in_=blobF_d[:, :])
            xjf = wt([128, N], "xjf")
            gdma(out=xjf, in_=bass.AP(tensor=srows_d.ap().tensor, offset=0,
                                      ap=[[0, 128], [1, N]]))
            yjf = wt([128, N], "yjf")
            gdma(out=yjf, in_=bass.AP(tensor=srows_d.ap().tensor, offset=N,
                                      ap=[[0, 128], [1, N]]))
            nb_bc = wt([SH, 12], "nbbc", F32)
            gdma(out=nb_bc, in_=bass.AP(tensor=nb2_d.ap().tensor, offset=0,
                                        ap=[[0, SH], [1, 12]]))

            hT = blobA[:, 0:256]
            aT = blobA[:, 256:512]
            movs = [aT, hT]
            wq_t = [blobW[:, GW_QA:GW_QA + EPg], blobW[:, GW_QH:GW_QH + EPg]]
            wk_t = [blobW[:, GW_KA:GW_KA + EPg], blobW[:, GW_KH:GW_KH + EPg]]
            wv_t = [blobW[:, GW_VA:GW_VA + E], blobW[:, GW_VH:GW_VH + E]]
            sel_t = [blobF[:, GF_SEL:GF_SEL + SH],
                     blobF[:, GF_SEL + SH:GF_SEL + 2 * SH]]
            wbig_t = [blobF[:, GF_WB + 6 * g:GF_WB + 6 * g + 6]
                      for g in range(8)]
            negx = [fside[:, 0:1], fside[:, 2:3]]
            negy = [fside[:, 1:2], fside[:, 3:4]]
            bq_t = [fside[:, GFS_BQ + h:GFS_BQ + h + 1] for h in range(4)] + \
                   [fside[0:64, GFS_BQ + 4:GFS_BQ + 5],
                    fside[0:64, GFS_BQ + 5:GFS_BQ + 6]]
            bk_t = [fside[:, GFS_BK + h:GFS_BK + h + 1] for h in range(4)] + \
                   [fside[0:64, GFS_BK + 4:GFS_BK + 5],
                    fside[0:64, GFS_BK + 5:GFS_BK + 6]]

            mask_t = []
            for c in range(2):
                bx = wt([128, N], f"bx{c}")
                by = wt([128, N], f"by{c}")
                nc.vector.tensor_scalar(bx, xjf, negx[c], None,
                                        mybir.AluOpType.add)
                nc.vector.tensor_scalar(by, yjf, negy[c], None,
                                        mybir.AluOpType.add)
                bx2 = wt([128, N], f"bx2{c}")
                by2 = wt([128, N], f"by2{c}")
                nc.vector.tensor_tensor(bx2, bx, bx, mybir.AluOpType.mult)
                nc.vector.tensor_tensor(by2, by, by, mybir.AluOpType.mult)
                nc.vector.tensor_scalar(bx, bx2, 16.0, None,
                                        mybir.AluOpType.is_le)
                nc.vector.tensor_scalar(by, by2, 4.0, None,
                                        mybir.AluOpType.is_le)
                prox = wt([128, N], f"prox{c}")
                nc.vector.tensor_tensor(prox, bx, by, mybir.AluOpType.mult)
                mk = wt([128, N], f"mask{c}")
                nc.gpsimd.affine_select(out=mk, in_=prox, pattern=[[1, N]],
                                        compare_op=mybir.AluOpType.is_gt,
                                        fill=0.0, base=-c * 128,
                                        channel_multiplier=-1)
                mask_t.append(mk)

            def proj(w_t, b_t, tag):
                outs = []
                for mi, (ms, ml) in enumerate(QKMg):
                    p = ps([ml, N])
                    for ci in range(2):
                        nc.tensor.matmul(p, w_t[ci][:, ms:ms + ml], movs[ci],
                                         start=(ci == 0), stop=(ci == 1))
                    t = wt([ml, N], f"{tag}{mi}")
                    nc.vector.tensor_scalar(t, p, b_t[mi], None,
                                            mybir.AluOpType.add)
                    outs.append(t)
                return outs

            qT = proj(wq_t, bq_t, "qT")
            kT = proj(wk_t, bk_t, "kT")

            E_t = [[None, None] for _ in range(NH)]
            ET_t = [[None, None] for _ in range(NH)]
            for h in range(NH):
                hs = [(h, 0, 128), (4 + h // 2, 32 * (h % 2), 32)]
                for mj in range(2):
                    pS = ps([128, N])
                    pST = ps([128, N])
                    for ci, (ti, rs, rl) in enumerate(hs):
                        st_, sp = (ci == 0), (ci == 1)
                        nc.tensor.matmul(
                            pS, qT[ti][rs:rs + rl, mj * 128:(mj + 1) * 128],
                            kT[ti][rs:rs + rl, :], start=st_, stop=sp)
                        nc.tensor.matmul(
                            pST, kT[ti][rs:rs + rl, mj * 128:(mj + 1) * 128],
                            qT[ti][rs:rs + rl, :], start=st_, stop=sp)
                    Eh = wt([128, N], f"E{h}_{mj}")
                    ETh = wt([128, N], f"ET{h}_{mj}")
                    nc.scalar.activation(Eh, pS,
                                         mybir.ActivationFunctionType.Exp,
                                         scale=SCALE)
                    nc.scalar.activation(ETh, pST,
                                         mybir.ActivationFunctionType.Exp,
                                         scale=SCALE)
                    E_t[h][mj] = Eh
                    ET_t[h][mj] = ETh

            v_t = []
            for nt in range(2):
                vt = wt([128, E], f"v{nt}")
                for ns, nl in ((0, 288), (288, 288)):
                    p = ps([128, nl])
                    for ci in range(2):
                        nc.tensor.matmul(
                            p, movs[ci][:, nt * 128:(nt + 1) * 128],
                            wv_t[ci][:, ns:ns + nl],
                            start=(ci == 0), stop=(ci == 1))
                    nc.any.tensor_copy(out=vt[:, ns:ns + nl], in_=p)
                v_t.append(vt)

            mcT = []
            for km in range(2):
                p = ps([128, SH])
                for c in range(2):
                    nc.tensor.matmul(
                        p, mask_t[c][:, km * 128:(km + 1) * 128],
                        sel_t[c], start=(c == 0), stop=(c == 1))
                t = wt([128, SH], f"mcT{km}")
                nc.any.tensor_copy(out=t, in_=p)
                mcT.append(t)
            ones_t = wt([128, 1], "ones_t")
            nc.vector.memset(ones_t, 1.0)
            pn = ps([SH, 1])
            for c in range(2):
                nc.tensor.matmul(pn, mcT[c], ones_t,
                                 start=(c == 0), stop=(c == 1))
            n_i = wt([SH, 1], "n_i", F32)
            nc.any.tensor_copy(out=n_i, in_=pn)

            RT = {}
            for h in range(NH):
                for jm in range(2):
                    p = ps([128, SH])
                    for kc in range(2):
                        nc.tensor.matmul(
                            p, ET_t[h][kc][:, jm * 128:(jm + 1) * 128],
                            mcT[kc], start=(kc == 0), stop=(kc == 1))
                    rtf = wt([128, SH], f"RTf{h}_{jm}", F32)
                    nc.vector.tensor_scalar(rtf, p, 1e-9, None,
                                            mybir.AluOpType.max)
                    with nc.allow_low_precision(reason="attn renorm"):
                        nc.vector.reciprocal(rtf, rtf)
                    rt = wt([128, SH], f"RT{h}_{jm}")
                    nc.vector.tensor_tensor(rt, rtf, mcT[jm],
                                            mybir.AluOpType.mult)
                    RT[(h, jm)] = rt
            WT = {}
            for h in range(NH):
                for km in range(2):
                    p = ps([128, SH])
                    for jc in range(2):
                        nc.tensor.matmul(
                            p, E_t[h][jc][:, km * 128:(km + 1) * 128],
                            RT[(h, jc)], start=(jc == 0), stop=(jc == 1))
                    wtl = wt([128, SH], f"WT{h}_{km}")
                    nc.vector.tensor_tensor(wtl, p, mcT[km],
                                            mybir.AluOpType.mult)
                    WT[(h, km)] = wtl
            ctxT8 = [None] * 8
            for h in range(NH):
                for dm, (ds, dl) in enumerate([(0, 128), (128, 16)]):
                    p = ps([dl, SH])
                    for kc in range(2):
                        nc.tensor.matmul(
                            p, v_t[kc][:, HD * h + ds:HD * h + ds + dl],
                            WT[(h, kc)], start=(kc == 0), stop=(kc == 1))
                    t = wt([dl, SH], f"cT{2 * h + dm}")
                    nc.any.tensor_copy(out=t, in_=p)
                    ctxT8[2 * h + dm] = t

            pVA = ps([SH, 6])
            for g in range(8):
                dl = 128 if g % 2 == 0 else 16
                nc.tensor.matmul(pVA, ctxT8[g], wbig_t[g][0:dl, :],
                                 start=(g == 0), stop=(g == 7))
            VAt = wt([SH, 6], "VAt", F32)
            nc.vector.scalar_tensor_tensor(
                out=VAt, in0=nb_bc[:, 0:6], scalar=n_i, in1=pVA,
                op0=mybir.AluOpType.mult, op1=mybir.AluOpType.add)
            VA = wt([SH, 6], "VA", F32)
            nc.vector.tensor_tensor(VA, VAt, nb_bc[:, 6:12],
                                    mybir.AluOpType.add)
            sA = wt([SH, 1], "sA", F32)
            nc.vector.reduce_sum(sA, VA[:, 1:6], axis=mybir.AxisListType.X)
            vm = wt([SH, 1], "vm", F32)
            nc.vector.scalar_tensor_tensor(out=vm, in0=sA, scalar=-0.2,
                                           in1=VA[:, 0:1],
                                           op0=mybir.AluOpType.mult,
                                           op1=mybir.AluOpType.add)
            Q_sb = wt([SH, ACT], "Qsb", F32)
            nc.vector.tensor_scalar(Q_sb, VA[:, 1:6], vm, None,
                                    mybir.AluOpType.add)
            nc.gpsimd.dma_start(out=out_d[:, :], in_=Q_sb)

    nc.compile()
    return nc


def _make_in_maps_general(inputs):
    f32 = np.float32
    g = lambda k: np.asarray(inputs[k], dtype=f32)

    hidden, action = g("hidden_state_n"), g("action_n")
    state = np.asarray(inputs["state_n"]).astype(np.int32)
    W_enc, b_enc = g("W_enc"), g("b_enc")
    Wiq, Wik, Wiv = g("Wiq"), g("Wik"), g("Wiv")

    def fuse(Wo_, bo_, Wi_, bi_):
        Wf = Wo_ @ Wi_
        WA = Wf[16:144]
        Wh = W_enc @ Wf[0:16]
        bf = b_enc @ Wf[0:16] + bo_ @ Wi_ + bi_
        return WA, Wh, bf

    WqA, Wqh, bqf = fuse(g("Wq"), g("bq"), Wiq, g("biq"))
    WkA, Wkh, bkf = fuse(g("Wk"), g("bk"), Wik, g("bik"))
    WvA, Wvh, bvf = fuse(g("Wv"), g("bv"), Wiv, g("biv"))

    Wva6 = np.concatenate([g("W_val").reshape(D, 1),
                           g("W_adv").reshape(D, ACT)], axis=1)
    WoWO = g("Wo_proj") @ g("W_O")
    Wbig = WoWO @ Wva6
    nvec = bvf @ Wbig + (g("bo_proj") @ g("W_O")) @ Wva6
    bva6 = np.concatenate([g("b_val").reshape(1), g("b_adv")])

    def padqk(w):
        mains = [w[:, 144 * h:144 * h + 128] for h in range(4)]
        z = np.zeros((w.shape[0], 16), f32)
        tails = [np.concatenate([w[:, 144 * h + 128:144 * h + 144], z,
                                 w[:, 144 * (h + 1) + 128:144 * (h + 1) + 144],
                                 z], axis=1) for h in (0, 2)]
        return np.concatenate(mains + tails, axis=1)

    def bias_cols(b):
        cols = np.zeros((128, 6), f32)
        for h in range(4):
            cols[:, h] = b[144 * h:144 * h + 128]
        for t, h in enumerate((0, 2)):
            cols[0:16, 4 + t] = b[144 * h + 128:144 * h + 144]
            cols[32:48, 4 + t] = b[144 * (h + 1) + 128:144 * (h + 1) + 144]
        return cols

    blobA = np.concatenate([np.ascontiguousarray(hidden.T),
                            np.ascontiguousarray(action.T)], axis=1)
    blobW = np.concatenate([padqk(WqA), padqk(Wqh), padqk(WkA), padqk(Wkh),
                            WvA, Wvh], axis=1)
    state_f = state.astype(f32)
    fside = np.zeros((128, GFS_COLS), f32)
    fside[:, 0] = -state_f[0:128, 0]
    fside[:, 1] = -state_f[0:128, 1]
    fside[:, 2] = -state_f[128:256, 0]
    fside[:, 3] = -state_f[128:256, 1]
    fside[:, GFS_BQ:GFS_BQ + 6] = bias_cols(bqf)
    fside[:, GFS_BK:GFS_BK + 6] = bias_cols(bkf)
    srows = np.ascontiguousarray(state_f.T)
    nb2 = np.concatenate([nvec, bva6]).reshape(1, 12).astype(f32)

    wbig8 = np.zeros((128, 48), f32)
    for h in range(4):
        wbig8[:, 12 * h:12 * h + 6] = Wbig[144 * h:144 * h + 128]
        wbig8[0:16, 12 * h + 6:12 * h + 12] = Wbig[144 * h + 128:144 * (h + 1)]

    eye = np.eye(N, dtype=f32)
    shared = {
        "blobA": blobA.astype(BF),
        "blobW": blobW.astype(BF),
        "fside": fside,
        "srows": srows.astype(BF),
        "nb2": nb2,
    }
    in_maps = []
    for c in range(NCORES):
        sel = eye[:, c * SH:(c + 1) * SH]
        selpack = np.concatenate([sel[0:128], sel[128:256]], axis=1)
        bF = np.concatenate([selpack, wbig8], axis=1)
        m = dict(shared)
        m["blobF"] = np.ascontiguousarray(bF).astype(BF)
        in_maps.append(m)
    return in_maps


def _kernel_general(inputs):
    if "gen" not in _NC_CACHE:
        _NC_CACHE["gen"] = _build_general()
    nc = _NC_CACHE["gen"]
    in_maps = _make_in_maps_general(inputs)
    res = bass_utils.run_bass_kernel_spmd(nc, in_maps,
                                          core_ids=list(range(NCORES)))
    return np.concatenate([res.results[c]["out"] for c in range(NCORES)],
                          axis=0)
